# revision 5
# baseline (speedup 1.0000x reference)
"""Fused pre-LN + QKV + attention + post-LN + residual kernel for TRN2.

Problem (nn_Attention_86517821210894):
    B=2, N=4096, C=512, H=8, D=64
    xn  = LN(x) ; qkv = xn @ w_qkv + b ; per-(b,h) softmax attention
    val = LN(attn_out) ; out = xn + val

Sharding (8 cores, zero collectives):
    core c -> batch b = c // 4, query-row block r = c % 4 (1024 rows).
    Each core receives the full x[b] (to build K/V for all 4096 keys) plus
    its own 1024-row query slice, and produces out[b, r*1024:(r+1)*1024].
    The K/V qkv matmul is recomputed by the 4 cores of a batch; this trades
    ~30% extra PE work for zero inter-core communication.

Device algorithm per core:
    1. pre-LN row-major (bn_stats/bn_aggr), PE-transpose into xnT [C, N] bf16
    2. qkv matmuls from xnT:  kT [2h*64, N], qT [2h*64, 1024], v [N, 8, 65]
       (v gets a ones column so the attention matmul also produces the
        softmax denominator)
    3. scoresT[k, q] = kT.T-slices @ qT (two heads packed via partition
       row-tiling), exp on ScalarE straight out of PSUM (softmax max-
       subtraction is skipped: scores*0.125 has |z| < ~2 for LN'd inputs,
       exp is safely in range; softmax is shift-invariant so the result
       matches the reference)
    4. AV: valT_aug[65, q] accumulated over key chunks in PSUM
    5. transpose valT back row-major, divide by the denominator column,
       post-LN, add xn residual, DMA out.
"""

import sys

sys.path.insert(0, "/opt/trn_rl_repo")

import numpy as np

B, N, C, H = 2, 4096, 512, 8
D = C // H
QR = N // 4  # query rows per core
EPS = 1e-5
SCALE = float(D) ** -0.5

_CACHE = {}


def _build(flags):
    (use_g_pre, use_beta_pre, use_g_post, use_beta_post, use_b_q, use_b_v) = flags

    import concourse.bacc as bacc
    import concourse.bass as bass
    import concourse.tile as tile
    from concourse import mybir
    from concourse.masks import make_identity

    f32 = mybir.dt.float32
    bf16 = mybir.dt.bfloat16
    AF = mybir.ActivationFunctionType
    ALU = mybir.AluOpType

    nc = bacc.Bacc(
        "TRN2", target_bir_lowering=False, debug=False, enable_asserts=False
    )

    xb = nc.dram_tensor("xb", [N, C], f32, kind="ExternalInput").ap()
    xq = nc.dram_tensor("xq", [QR, C], f32, kind="ExternalInput").ap()
    w = nc.dram_tensor("w_qkv", [C, 3 * C], bf16, kind="ExternalInput").ap()
    bqkv = nc.dram_tensor("b_qkv", [3 * C], f32, kind="ExternalInput").ap()
    g_pre = nc.dram_tensor("g_pre", [C], f32, kind="ExternalInput").ap()
    beta_pre = nc.dram_tensor("beta_pre", [C], f32, kind="ExternalInput").ap()
    g_post = nc.dram_tensor("g_post", [C], f32, kind="ExternalInput").ap()
    beta_post = nc.dram_tensor("beta_post", [C], f32, kind="ExternalInput").ap()
    out = nc.dram_tensor("out", [QR, C], f32, kind="ExternalOutput").ap()

    NT = N // 128  # 32 row tiles of x[b]
    QT = QR // 128  # 8 row tiles of the query block
    CCH = C // 128  # 4 contraction chunks
    KC = N // 128  # 32 key chunks

    def bcast(vec_ap, p, n):
        # replicate a [n] DRAM vector across p partitions
        return bass.AP(
            tensor=vec_ap.tensor,
            offset=vec_ap.offset,
            ap=[[0, p], *vec_ap.ap],
        )

    with tile.TileContext(nc) as tc:
        with (
            tc.tile_pool(name="consts", bufs=1) as consts,
            tc.tile_pool(name="ln_in", bufs=3) as ln_in,
            tc.tile_pool(name="stats", bufs=6) as stats,
            tc.tile_pool(name="xnrow", bufs=1) as xnrow_pool,
            tc.tile_pool(name="xnT", bufs=1) as xnT_pool,
            tc.tile_pool(name="xqT", bufs=1) as xqT_pool,
            tc.tile_pool(name="vsb", bufs=1) as v_pool,
            tc.tile_pool(name="kT", bufs=2) as kT_pool,
            tc.tile_pool(name="qT", bufs=2) as qT_pool,
            tc.tile_pool(name="expT", bufs=4) as expT_pool,
            tc.tile_pool(name="valT", bufs=2) as valT_pool,
            tc.tile_pool(name="valasm", bufs=1) as val_pool,
            tc.tile_pool(name="outp", bufs=2) as out_pool,
            tc.tile_pool(name="psc", bufs=2, space="PSUM") as psum_sc,
            tc.tile_pool(name="pqkv", bufs=2, space="PSUM") as psum_qkv,
            tc.tile_pool(name="pav", bufs=2, space="PSUM") as psum_av,
        ):
            # ---- constants ----
            ident = consts.tile([128, 128], f32)
            make_identity(nc, ident)
            eps_t = consts.tile([128, 1], f32)
            nc.vector.memset(eps_t, EPS)

            w_sb = consts.tile([128, CCH, 3 * C], bf16)
            nc.sync.dma_start(
                out=w_sb, in_=w.rearrange("(cc p) m -> p cc m", p=128)
            )

            g_pre_t = beta_pre_t = g_post_t = beta_post_t = None
            if use_g_pre:
                g_pre_t = consts.tile([128, C], f32)
                nc.sync.dma_start(out=g_pre_t, in_=bcast(g_pre, 128, C))
            if use_beta_pre:
                beta_pre_t = consts.tile([128, C], f32)
                nc.sync.dma_start(out=beta_pre_t, in_=bcast(beta_pre, 128, C))
            if use_g_post:
                g_post_t = consts.tile([128, C], f32)
                nc.sync.dma_start(out=g_post_t, in_=bcast(g_post, 128, C))
            if use_beta_post:
                beta_post_t = consts.tile([128, C], f32)
                nc.sync.dma_start(out=beta_post_t, in_=bcast(beta_post, 128, C))
            bq_t = None
            if use_b_q:
                # per-partition bias for qT tiles, one [128,1] column per pair
                bq_t = consts.tile([128, CCH, 1], f32)
                nc.sync.dma_start(
                    out=bq_t, in_=bqkv[0:C].rearrange("(cc p) -> p cc 1", p=128)
                )
            bv_t = None
            if use_b_v:
                bv_t = consts.tile([128, C], f32)
                nc.sync.dma_start(out=bv_t, in_=bcast(bqkv[2 * C : 3 * C], 128, C))

            # ---- persistent tensors ----
            xn_rows = xnrow_pool.tile([128, QT, C], f32)  # LN'd query rows
            xnT = xnT_pool.tile([128, CCH, N], bf16)  # LN(x[b]) transposed
            xqT = xqT_pool.tile([128, CCH, QR], bf16)  # query-row slice of xnT
            v_sb = v_pool.tile([128, KC, H, D + 1], bf16)  # v + ones column
            val_asm = val_pool.tile([128, QT, H, D + 1], f32)

            nc.vector.memset(v_sb[:, :, :, D : D + 1], 1.0)

            # ---- layernorm helper (row-major [128, C] f32) ----
            def layernorm(dst, src, g_t, beta_t):
                st = stats.tile([128, 6], f32, tag="bn6")
                nc.vector.bn_stats(out=st, in_=src)
                mv = stats.tile([128, 2], f32, tag="mv")
                nc.vector.bn_aggr(out=mv, in_=st)
                rstd = stats.tile([128, 1], f32, tag="rstd")
                nc.scalar.activation(
                    out=rstd, in_=mv[:, 1:2], func=AF.Sqrt, bias=eps_t, scale=1.0
                )
                nc.vector.reciprocal(out=rstd, in_=rstd)
                nc.vector.tensor_scalar(
                    out=dst,
                    in0=src,
                    scalar1=mv[:, 0:1],
                    scalar2=rstd,
                    op0=ALU.subtract,
                    op1=ALU.mult,
                )
                if g_t is not None:
                    nc.vector.tensor_mul(out=dst, in0=dst, in1=g_t)
                if beta_t is not None:
                    nc.vector.tensor_add(out=dst, in0=dst, in1=beta_t)

            # transpose a [128, C] f32 tile into four [128,128] blocks and
            # store (cast) into dstT[:, cc, col0:col0+128]
            def transpose_into(dstT, src, col0):
                ps = psum_sc.tile([128, 2, 512], f32, tag="psc")
                for cc in range(CCH):
                    nc.tensor.transpose(
                        ps[:, 0, :].rearrange("p (c n) -> p c n", n=128)[:, cc, :],
                        src[:, cc * 128 : (cc + 1) * 128],
                        ident,
                    )
                nc.vector.tensor_copy(
                    out=dstT[:, :, col0 : col0 + 128],
                    in_=ps[:, 0, :].rearrange("p (c n) -> p c n", n=128),
                )

            # ---- phase 1: pre-LN + transpose ----
            for i in range(NT):
                xt = ln_in.tile([128, C], f32, tag="xt")
                nc.sync.dma_start(out=xt, in_=xb[i * 128 : (i + 1) * 128, :])
                layernorm(xt, xt, g_pre_t, beta_pre_t)
                transpose_into(xnT, xt, i * 128)
            for i in range(QT):
                xt = ln_in.tile([128, C], f32, tag="xt")
                nc.sync.dma_start(out=xt, in_=xq[i * 128 : (i + 1) * 128, :])
                layernorm(xn_rows[:, i, :], xt, g_pre_t, beta_pre_t)
                transpose_into(xqT, xn_rows[:, i, :], i * 128)

            # ---- phase 2: v = xn @ w_v  (row-major, all heads at once) ----
            for kc in range(KC):
                pv = psum_qkv.tile([128, 512], f32, tag="pqkv")
                for cc in range(CCH):
                    nc.tensor.matmul(
                        pv,
                        xnT[:, cc, kc * 128 : (kc + 1) * 128],
                        w_sb[:, cc, 2 * C : 3 * C],
                        start=(cc == 0),
                        stop=(cc == CCH - 1),
                    )
                dst = v_sb[:, kc, :, 0:D]
                src = pv.rearrange("p (h d) -> p h d", d=D)
                if use_b_v:
                    nc.vector.tensor_add(
                        out=dst,
                        in0=src,
                        in1=bv_t.rearrange("p (h d) -> p h d", d=D),
                    )
                else:
                    nc.vector.tensor_copy(out=dst, in_=src)

            # ---- phases 3+4: per head-pair attention ----
            for pair in range(4):
                h_lo, h_hi = 2 * pair, 2 * pair + 1

                # kT for the pair: [128 (2 heads x 64), N]
                kT = kT_pool.tile([128, N], bf16, tag="kT")
                for rc in range(N // 512):
                    pk = psum_qkv.tile([128, 512], f32, tag="pqkv")
                    for cc in range(CCH):
                        nc.tensor.matmul(
                            pk,
                            w_sb[:, cc, C + pair * 128 : C + (pair + 1) * 128],
                            xnT[:, cc, rc * 512 : (rc + 1) * 512],
                            start=(cc == 0),
                            stop=(cc == CCH - 1),
                        )
                    nc.vector.tensor_copy(
                        out=kT[:, rc * 512 : (rc + 1) * 512], in_=pk
                    )

                # qT for the pair: [128, QR]
                qT = qT_pool.tile([128, QR], bf16, tag="qT")
                for rc in range(QR // 512):
                    pq = psum_qkv.tile([128, 512], f32, tag="pqkv")
                    for cc in range(CCH):
                        nc.tensor.matmul(
                            pq,
                            w_sb[:, cc, pair * 128 : (pair + 1) * 128],
                            xqT[:, cc, rc * 512 : (rc + 1) * 512],
                            start=(cc == 0),
                            stop=(cc == CCH - 1),
                        )
                        # q bias is per out-partition; add during copy
                    if use_b_q:
                        nc.vector.tensor_scalar_add(
                            out=qT[:, rc * 512 : (rc + 1) * 512],
                            in0=pq,
                            scalar1=bq_t[:, pair, :],
                        )
                    else:
                        nc.vector.tensor_copy(
                            out=qT[:, rc * 512 : (rc + 1) * 512], in_=pq
                        )

                for qb in range(QR // 512):
                    pav_lo = psum_av.tile([128, 512], f32, tag="pav")
                    pav_hi = psum_av.tile([128, 512], f32, tag="pav")
                    for kc in range(KC):
                        ps = psum_sc.tile([128, 2, 512], f32, tag="psc")
                        # two heads packed in the PE array (row groups 0-63 /
                        # 64-127), outputs side by side in one 2-bank tile
                        nc.tensor.matmul(
                            ps[:, 0, :],
                            kT[0:64, kc * 128 : (kc + 1) * 128],
                            qT[0:64, qb * 512 : (qb + 1) * 512],
                        )
                        nc.tensor.matmul(
                            ps[:, 1, :],
                            kT[64:128, kc * 128 : (kc + 1) * 128],
                            qT[64:128, qb * 512 : (qb + 1) * 512],
                        )
                        ex = expT_pool.tile([128, 2, 512], bf16, tag="expT")
                        nc.scalar.activation(
                            out=ex, in_=ps, func=AF.Exp, scale=SCALE
                        )
                        nc.tensor.matmul(
                            pav_lo[0 : D + 1, :],
                            v_sb[:, kc, h_lo, :],
                            ex[:, 0, :],
                            start=(kc == 0),
                            stop=(kc == KC - 1),
                        )
                        nc.tensor.matmul(
                            pav_hi[0 : D + 1, :],
                            v_sb[:, kc, h_hi, :],
                            ex[:, 1, :],
                            start=(kc == 0),
                            stop=(kc == KC - 1),
                        )
                    for h_idx, pav in ((0, pav_lo), (1, pav_hi)):
                        h = 2 * pair + h_idx
                        vt = valT_pool.tile([D + 1, 512], f32, tag="valT")
                        nc.vector.tensor_copy(out=vt, in_=pav[0 : D + 1, :])
                        for j in range(4):
                            pt = psum_sc.tile([128, 2, 512], f32, tag="psc")
                            nc.tensor.transpose(
                                pt[0:128, 0, 0 : D + 1],
                                vt[:, j * 128 : (j + 1) * 128],
                                ident[0 : D + 1, 0 : D + 1],
                            )
                            qtile = qb * 4 + j
                            nc.vector.tensor_copy(
                                out=val_asm[:, qtile, h, :],
                                in_=pt[0:128, 0, 0 : D + 1],
                            )

            # ---- phase 5: normalize, post-LN, residual, store ----
            for qtile in range(QT):
                va = val_asm[:, qtile]
                ot = out_pool.tile([128, C], f32, tag="ot")
                for h in range(H):
                    rs = stats.tile([128, 1], f32, tag="rs")
                    nc.vector.reciprocal(out=rs, in_=va[:, h, D : D + 1])
                    nc.vector.tensor_scalar_mul(
                        out=ot[:, h * D : (h + 1) * D],
                        in0=va[:, h, 0:D],
                        scalar1=rs,
                    )
                if use_b_v:
                    nc.vector.tensor_add(out=ot, in0=ot, in1=bv_t)
                layernorm(ot, ot, g_post_t, beta_post_t)
                nc.vector.tensor_add(out=ot, in0=ot, in1=xn_rows[:, qtile, :])
                nc.sync.dma_start(
                    out=out[qtile * 128 : (qtile + 1) * 128, :], in_=ot
                )

    nc.compile()
    return nc


def kernel(x, w_qkv, b_qkv, g_pre, beta_pre, g_post, beta_post):
    import ml_dtypes
    from concourse.bass_utils import run_bass_kernel_spmd

    x = np.asarray(x, dtype=np.float32)
    w_qkv = np.asarray(w_qkv, dtype=np.float32)
    b_qkv = np.asarray(b_qkv, dtype=np.float32)
    g_pre = np.asarray(g_pre, dtype=np.float32)
    beta_pre = np.asarray(beta_pre, dtype=np.float32)
    g_post = np.asarray(g_post, dtype=np.float32)
    beta_post = np.asarray(beta_post, dtype=np.float32)

    flags = (
        not np.all(g_pre == 1.0),
        not np.all(beta_pre == 0.0),
        not np.all(g_post == 1.0),
        not np.all(beta_post == 0.0),
        not np.all(b_qkv[0:C] == 0.0),
        not np.all(b_qkv[2 * C : 3 * C] == 0.0),
    )
    # NOTE: b_qkv[C:2C] (the K bias) provably cancels in softmax and is
    # intentionally never applied.
    if flags not in _CACHE:
        _CACHE[flags] = _build(flags)
    nc = _CACHE[flags]

    w_bf = w_qkv.astype(ml_dtypes.bfloat16)
    in_maps = []
    for c in range(8):
        b = c // 4
        r = c % 4
        in_maps.append(
            {
                "xb": np.ascontiguousarray(x[b]),
                "xq": np.ascontiguousarray(x[b, r * QR : (r + 1) * QR]),
                "w_qkv": w_bf,
                "b_qkv": b_qkv,
                "g_pre": g_pre,
                "beta_pre": beta_pre,
                "g_post": g_post,
                "beta_post": beta_post,
            }
        )

    global _last_in_maps
    _last_in_maps = in_maps
    res = run_bass_kernel_spmd(nc, in_maps, core_ids=list(range(8)))
    out = np.empty((B, N, C), dtype=np.float32)
    for c in range(8):
        b = c // 4
        r = c % 4
        out[b, r * QR : (r + 1) * QR] = res.results[c]["out"]
    return out


# revision 7
# speedup vs baseline: 1.1347x; 1.1347x over previous
"""Fused pre-LN + QKV + attention + post-LN + residual kernel for TRN2.

Problem (nn_Attention_86517821210894):
    B=2, N=4096, C=512, H=8, D=64
    xn  = LN(x) ; qkv = xn @ w_qkv + b ; per-(b,h) softmax attention
    val = LN(attn_out) ; out = xn + val

Sharding (8 cores, zero collectives):
    core c -> batch b = c // 4, query-row block r = c % 4 (1024 rows).
    Each core receives x[b] ROTATED so its query block is rows 0:1024
    (softmax and the value sum are permutation-invariant over keys, so
    rotating the key order changes nothing), builds K/V for all 4096
    keys, and produces out[b, r*1024:(r+1)*1024].  The K/V qkv matmul is
    recomputed by the 4 cores of a batch; this trades ~30% extra PE work
    for zero inter-core communication.

Device algorithm per core (single fused pipeline; Tile's range-granular
dependences let kT/v/scores chase the pre-LN transposes column by column):
    1. pre-LN row-major (bn_stats/bn_aggr), PE-transpose into xnT [C, N] bf16
    2. qkv matmuls from xnT:  kT [2h*64, N] per pair, qT [2h*64, 1024],
       v [N, 8, 65] (ones column -> the AV matmul also produces the
       softmax denominator)
    3. scoresT[k, q] = kT-slice.T @ qT-slice with two heads packed in the
       PE array (partition row groups 0-63 / 64-127); exp on ScalarE
       straight out of PSUM in 3-bank groups (max-subtraction is skipped:
       scores*0.125 has |z| < ~2 for LN'd inputs, exp is safely in range;
       softmax is shift-invariant so the result matches the reference)
    4. AV: valT_aug[65, q] accumulated over key chunks in PSUM
    5. transpose valT row-major, divide by the denominator column,
       post-LN (single batched Sqrt at the end to avoid ACT table
       thrash), add xn residual, DMA out.
"""

import sys

sys.path.insert(0, "/opt/trn_rl_repo")

import numpy as np

B, N, C, H = 2, 4096, 512, 8
D = C // H
QR = N // 4  # query rows per core
EPS = 1e-5
SCALE = float(D) ** -0.5

_CACHE = {}


def _build(flags):
    (use_g_pre, use_beta_pre, use_g_post, use_beta_post, use_b_q, use_b_v) = flags

    import concourse.bacc as bacc
    import concourse.bass as bass
    import concourse.tile as tile
    from concourse import mybir
    from concourse.masks import make_identity

    f32 = mybir.dt.float32
    bf16 = mybir.dt.bfloat16
    AF = mybir.ActivationFunctionType
    ALU = mybir.AluOpType

    nc = bacc.Bacc(
        "TRN2", target_bir_lowering=False, debug=False, enable_asserts=False
    )

    xb = nc.dram_tensor("xb", [N, C], f32, kind="ExternalInput").ap()
    w = nc.dram_tensor("w_qkv", [C, 3 * C], bf16, kind="ExternalInput").ap()
    bqkv = nc.dram_tensor("b_qkv", [3 * C], f32, kind="ExternalInput").ap()
    g_pre = nc.dram_tensor("g_pre", [C], f32, kind="ExternalInput").ap()
    beta_pre = nc.dram_tensor("beta_pre", [C], f32, kind="ExternalInput").ap()
    g_post = nc.dram_tensor("g_post", [C], f32, kind="ExternalInput").ap()
    beta_post = nc.dram_tensor("beta_post", [C], f32, kind="ExternalInput").ap()
    out = nc.dram_tensor("out", [QR, C], f32, kind="ExternalOutput").ap()

    NT = N // 128  # 32 row tiles of x[b]
    QT = QR // 128  # 8 row tiles of the query block
    CCH = C // 128  # 4 contraction chunks
    KC = N // 128  # 32 key chunks
    NPAIR = H // 2

    def bcast(vec_ap, p):
        return bass.AP(
            tensor=vec_ap.tensor, offset=vec_ap.offset, ap=[[0, p], *vec_ap.ap]
        )

    with tile.TileContext(nc) as tc:
        with (
            tc.tile_pool(name="consts", bufs=1) as consts,
            tc.tile_pool(name="ln_in", bufs=4) as ln_in,
            tc.tile_pool(name="stats", bufs=8) as stats,
            tc.tile_pool(name="xnrow", bufs=1) as xnrow_pool,
            tc.tile_pool(name="xnT", bufs=1) as xnT_pool,
            tc.tile_pool(name="vsb", bufs=1) as v_pool,
            tc.tile_pool(name="kT", bufs=2) as kT_pool,
            tc.tile_pool(name="qT", bufs=2) as qT_pool,
            tc.tile_pool(name="expT", bufs=4) as expT_pool,
            tc.tile_pool(name="valT", bufs=2) as valT_pool,
            tc.tile_pool(name="valasm", bufs=1) as val_pool,
            tc.tile_pool(name="outp", bufs=2) as out_pool,
            tc.tile_pool(name="ps3", bufs=2, space="PSUM") as ps3,
            tc.tile_pool(name="pav", bufs=2, space="PSUM") as psum_av,
        ):
            # ---- warmup burst: get the PE HAM to K=8/8 immediately ----
            dummy = consts.tile([128, 512], bf16)
            nc.gpsimd.memset(dummy, 0.0)
            pw = ps3.tile([128, 3, 512], f32, tag="ps3")
            for i in range(16):
                nc.tensor.matmul(pw[:, 0, :], dummy[:, 0:128], dummy)
            del pw

            # ---- constants ----
            ident = consts.tile([128, 128], f32)
            make_identity(nc, ident)
            eps_t = consts.tile([128, 1], f32)
            nc.vector.memset(eps_t, EPS)

            w_sb = consts.tile([128, CCH, 3 * C], bf16)
            nc.sync.dma_start(out=w_sb, in_=w.rearrange("(cc p) m -> p cc m", p=128))

            g_pre_t = beta_pre_t = g_post_t = beta_post_t = None
            if use_g_pre:
                g_pre_t = consts.tile([128, C], f32)
                nc.sync.dma_start(out=g_pre_t, in_=bcast(g_pre, 128))
            if use_beta_pre:
                beta_pre_t = consts.tile([128, C], f32)
                nc.sync.dma_start(out=beta_pre_t, in_=bcast(beta_pre, 128))
            if use_g_post:
                g_post_t = consts.tile([128, C], f32)
                nc.sync.dma_start(out=g_post_t, in_=bcast(g_post, 128))
            if use_beta_post:
                beta_post_t = consts.tile([128, C], f32)
                nc.sync.dma_start(out=beta_post_t, in_=bcast(beta_post, 128))
            bq_t = None
            if use_b_q:
                bq_t = consts.tile([128, CCH, 1], f32)
                nc.sync.dma_start(
                    out=bq_t, in_=bqkv[0:C].rearrange("(cc p) -> p cc 1", p=128)
                )
            bv_t = None
            if use_b_v:
                bv_t = consts.tile([128, C], f32)
                nc.sync.dma_start(out=bv_t, in_=bcast(bqkv[2 * C : 3 * C], 128))

            # ---- persistent tensors ----
            xn_rows = xnrow_pool.tile([128, QT, C], f32)
            xnT = xnT_pool.tile([128, CCH, N], bf16)
            v_sb = v_pool.tile([128, KC, H, D + 1], bf16)
            val_asm = val_pool.tile([128, QT, H, D + 1], f32)
            means = stats.tile([128, QT], f32, tag="means", bufs=1)
            rstds = stats.tile([128, QT], f32, tag="rstds", bufs=1)

            nc.vector.memset(v_sb[:, :, :, D : D + 1], 1.0)

            def layernorm(dst, src, g_t, beta_t):
                st = stats.tile([128, 6], f32, tag="bn6")
                nc.vector.bn_stats(out=st, in_=src)
                mv = stats.tile([128, 2], f32, tag="mv")
                nc.vector.bn_aggr(out=mv, in_=st)
                rstd = stats.tile([128, 1], f32, tag="rstd")
                nc.scalar.activation(
                    out=rstd, in_=mv[:, 1:2], func=AF.Sqrt, bias=eps_t, scale=1.0
                )
                nc.vector.reciprocal(out=rstd, in_=rstd)
                nc.vector.tensor_scalar(
                    out=dst,
                    in0=src,
                    scalar1=mv[:, 0:1],
                    scalar2=rstd,
                    op0=ALU.subtract,
                    op1=ALU.mult,
                )
                if g_t is not None:
                    nc.vector.tensor_mul(out=dst, in0=dst, in1=g_t)
                if beta_t is not None:
                    nc.vector.tensor_add(out=dst, in0=dst, in1=beta_t)

            def transpose_into(dstT, src, col0):
                ps = ps3.tile([128, 3, 512], f32, tag="ps3")
                pview = ps[:, 0, :].rearrange("p (c n) -> p c n", n=128)
                for cc in range(CCH):
                    nc.tensor.transpose(
                        pview[:, cc, :], src[:, cc * 128 : (cc + 1) * 128], ident
                    )
                nc.any.tensor_copy(out=dstT[:, :, col0 : col0 + 128], in_=pview)

            def produce_v(kc):
                pv = ps3.tile([128, 3, 512], f32, tag="ps3")
                for cc in range(CCH):
                    nc.tensor.matmul(
                        pv[:, 0, :],
                        xnT[:, cc, kc * 128 : (kc + 1) * 128],
                        w_sb[:, cc, 2 * C : 3 * C],
                        start=(cc == 0),
                        stop=(cc == CCH - 1),
                    )
                src = pv[:, 0, :].rearrange("p (h d) -> p h d", d=D)
                dst = v_sb[:, kc, :, 0:D]
                if use_b_v:
                    nc.any.tensor_add(
                        out=dst, in0=src, in1=bv_t.rearrange("p (h d) -> p h d", d=D)
                    )
                else:
                    nc.any.tensor_copy(out=dst, in_=src)

            def produce_kT(pair, kT, rc):
                pk = ps3.tile([128, 3, 512], f32, tag="ps3")
                for cc in range(CCH):
                    nc.tensor.matmul(
                        pk[:, 0, :],
                        w_sb[:, cc, C + pair * 128 : C + (pair + 1) * 128],
                        xnT[:, cc, rc * 512 : (rc + 1) * 512],
                        start=(cc == 0),
                        stop=(cc == CCH - 1),
                    )
                nc.any.tensor_copy(
                    out=kT[:, rc * 512 : (rc + 1) * 512], in_=pk[:, 0, :]
                )

            def produce_qT(pair, qT, rc):
                pq = ps3.tile([128, 3, 512], f32, tag="ps3")
                for cc in range(CCH):
                    nc.tensor.matmul(
                        pq[:, 0, :],
                        w_sb[:, cc, pair * 128 : (pair + 1) * 128],
                        xnT[:, cc, rc * 512 : (rc + 1) * 512],
                        start=(cc == 0),
                        stop=(cc == CCH - 1),
                    )
                if use_b_q:
                    nc.any.tensor_scalar_add(
                        out=qT[:, rc * 512 : (rc + 1) * 512],
                        in0=pq[:, 0, :],
                        scalar1=bq_t[:, pair, :],
                    )
                else:
                    nc.any.tensor_copy(
                        out=qT[:, rc * 512 : (rc + 1) * 512], in_=pq[:, 0, :]
                    )

            # ---- phase 1 + 2 interleaved: LN/transpose chased by kT/qT/v ----
            # (emitted per 512-column group so the scheduler can pipeline)
            kT0 = kT_pool.tile([128, N], bf16, tag="kT")
            qT0 = qT_pool.tile([128, QR], bf16, tag="qT")
            for rc in range(NT // 4):
                for j in range(4):
                    i = rc * 4 + j
                    xt = ln_in.tile([128, C], f32, tag="xt")
                    nc.sync.dma_start(out=xt, in_=xb[i * 128 : (i + 1) * 128, :])
                    if i < QT:
                        layernorm(xn_rows[:, i, :], xt, g_pre_t, beta_pre_t)
                        transpose_into(xnT, xn_rows[:, i, :], i * 128)
                    else:
                        layernorm(xt, xt, g_pre_t, beta_pre_t)
                        transpose_into(xnT, xt, i * 128)
                    produce_v(i)
                produce_kT(0, kT0, rc)
                if rc < QR // 512:
                    produce_qT(0, qT0, rc)

            # ---- attention per head pair ----
            def attention(pair, kT, qT):
                h_lo, h_hi = 2 * pair, 2 * pair + 1
                # scoresT slices (qb, kc, head), grouped 3 per psum tile so
                # each ScalarE exp covers 1536 elements/partition
                exp_of = {}  # (qb, kc, h_idx) -> (expT_tile, pos)
                group = None
                pos = 0
                pending = []

                def flush_group(g, n):
                    ex = expT_pool.tile([128, 3, 512], bf16, tag="expT")
                    nc.scalar.activation(
                        out=ex[:, 0:n, :], in_=g[:, 0:n, :], func=AF.Exp, scale=SCALE
                    )
                    for key, p in pending:
                        exp_of[key] = (ex, p)
                    pending.clear()

                for qb in range(QR // 512):
                    for kc in range(KC):
                        for h_idx in range(2):
                            if group is None:
                                group = ps3.tile([128, 3, 512], f32, tag="ps3")
                                pos = 0
                            base = h_idx * 64
                            nc.tensor.matmul(
                                group[:, pos, :],
                                kT[base : base + 64, kc * 128 : (kc + 1) * 128],
                                qT[base : base + 64, qb * 512 : (qb + 1) * 512],
                            )
                            pending.append(((qb, kc, h_idx), pos))
                            pos += 1
                            if pos == 3:
                                flush_group(group, 3)
                                group = None
                if group is not None:
                    flush_group(group, pos)
                    group = None

                # Now emit AV matmuls in slice order (deps on expT tiles are
                # tracked by Tile; PE will interleave them with the scores
                # stream automatically)
                for qb in range(QR // 512):
                    pav_lo = psum_av.tile([128, 512], f32, tag="pav")
                    pav_hi = psum_av.tile([128, 512], f32, tag="pav")
                    for kc in range(KC):
                        ex_lo, p_lo = exp_of[(qb, kc, 0)]
                        ex_hi, p_hi = exp_of[(qb, kc, 1)]
                        nc.tensor.matmul(
                            pav_lo[0 : D + 1, :],
                            v_sb[:, kc, h_lo, :],
                            ex_lo[:, p_lo, :],
                            start=(kc == 0),
                            stop=(kc == KC - 1),
                        )
                        nc.tensor.matmul(
                            pav_hi[0 : D + 1, :],
                            v_sb[:, kc, h_hi, :],
                            ex_hi[:, p_hi, :],
                            start=(kc == 0),
                            stop=(kc == KC - 1),
                        )
                    for h_idx, pv_ in ((0, pav_lo), (1, pav_hi)):
                        h = 2 * pair + h_idx
                        vt = valT_pool.tile([D + 1, 512], f32, tag="valT")
                        nc.vector.tensor_copy(out=vt, in_=pv_[0 : D + 1, :])
                        for j in range(4):
                            pt = ps3.tile([128, 3, 512], f32, tag="ps3")
                            nc.tensor.transpose(
                                pt[:, 0, 0 : D + 1],
                                vt[:, j * 128 : (j + 1) * 128],
                                ident[0 : D + 1, 0 : D + 1],
                            )
                            qtile = qb * 4 + j
                            nc.vector.tensor_copy(
                                out=val_asm[:, qtile, h, :],
                                in_=pt[:, 0, 0 : D + 1],
                            )

            attention(0, kT0, qT0)
            for pair in range(1, NPAIR):
                kT = kT_pool.tile([128, N], bf16, tag="kT")
                qT = qT_pool.tile([128, QR], bf16, tag="qT")
                for rc in range(N // 512):
                    produce_kT(pair, kT, rc)
                for rc in range(QR // 512):
                    produce_qT(pair, qT, rc)
                attention(pair, kT, qT)

            # ---- phase 5: denominator, post-LN (batched sqrt), residual ----
            ots = []
            for qtile in range(QT):
                va = val_asm[:, qtile]
                ot = out_pool.tile([128, C], f32, tag="ot", bufs=QT)
                for h in range(H):
                    rs = stats.tile([128, 1], f32, tag="rs")
                    nc.vector.reciprocal(out=rs, in_=va[:, h, D : D + 1])
                    nc.vector.tensor_scalar_mul(
                        out=ot[:, h * D : (h + 1) * D],
                        in0=va[:, h, 0:D],
                        scalar1=rs,
                    )
                if use_b_v:
                    nc.vector.tensor_add(out=ot, in0=ot, in1=bv_t)
                st = stats.tile([128, 6], f32, tag="bn6")
                nc.vector.bn_stats(out=st, in_=ot)
                mv = stats.tile([128, 2], f32, tag="mv")
                nc.vector.bn_aggr(out=mv, in_=st)
                nc.vector.tensor_copy(
                    out=means[:, qtile : qtile + 1], in_=mv[:, 0:1]
                )
                nc.vector.tensor_copy(
                    out=rstds[:, qtile : qtile + 1], in_=mv[:, 1:2]
                )
                ots.append(ot)
            nc.scalar.activation(
                out=rstds, in_=rstds, func=AF.Sqrt, bias=eps_t, scale=1.0
            )
            nc.vector.reciprocal(out=rstds, in_=rstds)
            for qtile in range(QT):
                ot = ots[qtile]
                nc.vector.tensor_scalar(
                    out=ot,
                    in0=ot,
                    scalar1=means[:, qtile : qtile + 1],
                    scalar2=rstds[:, qtile : qtile + 1],
                    op0=ALU.subtract,
                    op1=ALU.mult,
                )
                if use_g_post:
                    nc.vector.tensor_mul(out=ot, in0=ot, in1=g_post_t)
                if use_beta_post:
                    nc.vector.tensor_add(out=ot, in0=ot, in1=beta_post_t)
                nc.vector.tensor_add(out=ot, in0=ot, in1=xn_rows[:, qtile, :])
                nc.sync.dma_start(
                    out=out[qtile * 128 : (qtile + 1) * 128, :], in_=ot
                )

    nc.compile()
    return nc


def kernel(x, w_qkv, b_qkv, g_pre, beta_pre, g_post, beta_post):
    import ml_dtypes
    from concourse.bass_utils import run_bass_kernel_spmd

    x = np.asarray(x, dtype=np.float32)
    w_qkv = np.asarray(w_qkv, dtype=np.float32)
    b_qkv = np.asarray(b_qkv, dtype=np.float32)
    g_pre = np.asarray(g_pre, dtype=np.float32)
    beta_pre = np.asarray(beta_pre, dtype=np.float32)
    g_post = np.asarray(g_post, dtype=np.float32)
    beta_post = np.asarray(beta_post, dtype=np.float32)

    flags = (
        not np.all(g_pre == 1.0),
        not np.all(beta_pre == 0.0),
        not np.all(g_post == 1.0),
        not np.all(beta_post == 0.0),
        not np.all(b_qkv[0:C] == 0.0),
        not np.all(b_qkv[2 * C : 3 * C] == 0.0),
    )
    # NOTE: b_qkv[C:2C] (the K bias) provably cancels in softmax and is
    # intentionally never applied.
    if flags not in _CACHE:
        _CACHE[flags] = _build(flags)
    nc = _CACHE[flags]

    w_bf = w_qkv.astype(ml_dtypes.bfloat16)
    in_maps = []
    for c in range(8):
        b = c // 4
        r = c % 4
        xrot = np.ascontiguousarray(
            np.concatenate([x[b, r * QR :], x[b, : r * QR]], axis=0)
        )
        in_maps.append(
            {
                "xb": xrot,
                "w_qkv": w_bf,
                "b_qkv": b_qkv,
                "g_pre": g_pre,
                "beta_pre": beta_pre,
                "g_post": g_post,
                "beta_post": beta_post,
            }
        )

    global _last_in_maps
    _last_in_maps = in_maps
    res = run_bass_kernel_spmd(nc, in_maps, core_ids=list(range(8)))
    out = np.empty((B, N, C), dtype=np.float32)
    for c in range(8):
        b = c // 4
        r = c % 4
        out[b, r * QR : (r + 1) * QR] = res.results[c]["out"]
    return out
